# revision 17
# baseline (speedup 1.0000x reference)
"""AttentionPool (single CLS query over ragged segments) on 8 TRN2 NeuronCores.

v5 design (DMA-bound; the original version was PE-bound on device-side
transposes):
  - Host folds the CLS query into the key projection:
        wq[i, h] = softmax_scale * sum_{j in head h} cls[j] * W_k[j, i]
    so scores[t, h] = sum_i embed[t, i] * wq[i, h].  Key bias and softmax
    max-subtraction shift scores by a per-(head, segment) constant that
    cancels in softmax => omitted (|s| <~ 30 << 88, exp stays finite in f32).
  - Host pushes embed in BOTH layouts so the device never transposes x:
      xt: d-major bf16        (feeds the scores matmul directly)
      xn: token-major fp8e3m4 (the num matmul moving operand; e3m4's 4
          mantissa bits keep the pooled-output error ~1.8% < 2% budget for
          the canonical 2048-token segments; bf16 stationary x fp8 moving
          matmuls verified bit-exact on HW.  If any segment is shorter than
          1536 tokens the pooling averages fewer values and fp8 noise would
          grow, so the host falls back to bf16 xn automatically.)
    Both tensors are pre-tiled on the host into the exact SBUF layout of one
    512-token quarter so each DMA is a single contiguous multi-KB run per
    partition; quarters are prefetched 6 deep on two alternating hwdge
    queues so neither queue ever idles on buffer recycling.
  - Device per quarter: scores = wq.T @ xt tiles (PE, bf16), exp on ACT with
    denominator accumulation, p transposed token-major via DVE 32x32 stream
    transposes (PE untouched), num[h,:] += p_chunk.T @ xn_chunk accumulated
    in PSUM over the segment.  Scores of quarter i+1 are emitted before the
    num matmuls of quarter i so the PE never waits on the ACT-exp /
    DVE-transpose chain.
  - Device outputs raw num [H, D] and per-quarter denoms; the host does the
    final out[i] = num[head(i), i] / denom[head(i)] (trivial numpy).

Self-contained: hardcodes the problem shapes; handles arbitrary cu_lens by
padding each segment slot to a fixed chunk grid (masked), which degenerates
to zero overhead for the expected equal-length segmentation.
"""

import math

import numpy as np

H = 20        # heads
D = 1280      # embed dim
DH = D // H   # head dim (64)
P = 128       # partitions
DC = D // P   # 10 d-chunks
NCORES = 8
QCH = 4       # chunks per quarter (512 tokens)
QP = QCH * P  # tokens per quarter
LOOK = 4      # quarters of DMA prefetch


def _ceil_div(a, b):
    return -(-a // b)


def _build_program(S, K, use_mask, xn_lowp):
    """SPMD Bass program: S segment slots x K chunks x 128 tokens per core."""
    import concourse.tile as tile
    from concourse import bacc, mybir

    f32 = mybir.dt.float32
    bf16 = mybir.dt.bfloat16
    xn_dt = mybir.dt.float8e3 if xn_lowp else bf16
    Exp = mybir.ActivationFunctionType.Exp

    NQ = _ceil_div(K, QCH)         # quarters per slot
    L = NQ * QP                    # padded tokens per slot

    nc = bacc.Bacc()
    # pre-tiled on host: [slot, quarter, partition, flat SBUF bytes]
    xt = nc.dram_tensor("xt", [S, NQ, P, DC * QP], bf16, kind="ExternalInput")
    xn = nc.dram_tensor("xn", [S, NQ, P, QCH * D], xn_dt, kind="ExternalInput")
    wqd = nc.dram_tensor("wqd", [P, DC * H], bf16, kind="ExternalInput")
    maskin = None
    if use_mask:
        maskin = nc.dram_tensor("maskin", [S * L], f32, kind="ExternalInput")
    onum = nc.dram_tensor("onum", [S * H, D], f32, kind="ExternalOutput")
    oden = nc.dram_tensor("oden", [S * H, NQ], f32, kind="ExternalOutput")

    with tile.TileContext(nc) as tc:
        with tc.tile_pool(name="persist", bufs=1) as persist:
            wq_sb = persist.tile([P, DC, H], bf16)
            # contiguous 400B-per-partition load on the (otherwise idle at
            # start) gpsimd queue so it never delays the first xt tile
            nc.gpsimd.dma_start(
                out=wq_sb.rearrange("p dc h -> p (dc h)"), in_=wqd[:, :])

            with tc.tile_pool(name="xt", bufs=LOOK + 2) as xt_pool, \
                 tc.tile_pool(name="xn", bufs=LOOK + 2) as xn_pool, \
                 tc.tile_pool(name="pp", bufs=2) as pp_pool, \
                 tc.tile_pool(name="pt", bufs=2) as pt_pool, \
                 tc.tile_pool(name="pta", bufs=2) as pta_pool, \
                 tc.tile_pool(name="small", bufs=2) as small_pool, \
                 tc.tile_pool(name="ps_s", bufs=2, space="PSUM") as ps_s_pool, \
                 tc.tile_pool(name="ps_n", bufs=2, space="PSUM") as ps_n_pool:

                qtiles = {}  # (seg, q) -> (xt_q, xn_q)
                segst = {}   # seg -> (p_sb, pt_t, dens)
                pnums = {}   # seg -> psum num tile

                def alloc_tiles(seg, q):
                    xt_q = xt_pool.tile([P, DC, QP], bf16, tag="xt")
                    xn_q = xn_pool.tile([P, QCH, D], xn_dt, tag="xn")
                    qtiles[(seg, q)] = (xt_q, xn_q)
                    if q == 0:
                        p_sb = pp_pool.tile([32, L], bf16, tag="p")
                        pt_t = pt_pool.tile([P, K, 32], bf16, tag="pt")
                        dens = small_pool.tile([32, NQ], f32, tag="dens")
                        segst[seg] = (p_sb, pt_t, dens)

                def emit_quarter_loads(seg, q):
                    alloc_tiles(seg, q)
                    (xt_q, xn_q) = qtiles[(seg, q)]
                    qs = (nc.sync, nc.scalar)
                    qi = seg * NQ + q
                    qs[qi % 2].dma_start(
                        out=xt_q.rearrange("p dc t -> p (dc t)"),
                        in_=xt[seg, q, :, :])
                    qs[(qi + 1) % 2].dma_start(
                        out=xn_q.rearrange("p c i -> p (c i)"),
                        in_=xn[seg, q, :, :])

                def emit_warm_loads(jobs, nwarm):
                    # warmup: each tile split across both queues (full HBM
                    # bandwidth per tile) and issued in consumption order
                    # with xt one step ahead of xn, so the pipeline never
                    # waits on an out-of-order transfer
                    for j in range(nwarm):
                        alloc_tiles(*jobs[j])
                    order = []
                    for j in range(nwarm):
                        order.append(("xt", j))
                        if j >= 1:
                            order.append(("xn", j - 1))
                    order.append(("xn", nwarm - 1))
                    hd = DC // 2 * QP
                    hn = QCH // 2 * D
                    for kind, j in order:
                        seg, q = jobs[j]
                        (xt_q, xn_q) = qtiles[(seg, q)]
                        if kind == "xt":
                            flat = xt_q.rearrange("p dc t -> p (dc t)")
                            nc.sync.dma_start(
                                out=flat[:, 0:hd], in_=xt[seg, q, :, 0:hd])
                            nc.scalar.dma_start(
                                out=flat[:, hd:],
                                in_=xt[seg, q, :, hd:DC * QP])
                        else:
                            flat = xn_q.rearrange("p c i -> p (c i)")
                            nc.sync.dma_start(
                                out=flat[:, 0:hn], in_=xn[seg, q, :, 0:hn])
                            nc.scalar.dma_start(
                                out=flat[:, hn:],
                                in_=xn[seg, q, :, hn:QCH * D])

                def emit_scores(seg, q):
                    (xt_q, xn_q) = qtiles[(seg, q)]
                    (p_sb, pt_t, dens) = segst[seg]
                    qc = min(QCH, K - q * QCH)
                    cols = qc * P
                    off = q * QP              # token offset within slot
                    sc = ps_s_pool.tile([H, QP], f32, tag="sc")
                    for dc in range(DC):
                        nc.tensor.matmul(
                            sc[:, :cols],
                            lhsT=wq_sb[:, dc, :],
                            rhs=xt_q[:, dc, 0:cols],
                            start=(dc == 0), stop=(dc == DC - 1))
                    # exp (h-major) + denominator
                    if use_mask:
                        nc.scalar.activation(
                            out=p_sb[0:H, off:off + cols], in_=sc[:, :cols],
                            func=Exp)
                        msk = small_pool.tile([H, QP], f32, tag="msk")
                        nc.gpsimd.dma_start(
                            out=msk[:, :cols],
                            in_=maskin[seg * L + off:seg * L + off + cols]
                            .partition_broadcast(H))
                        nc.vector.tensor_mul(
                            p_sb[0:H, off:off + cols],
                            p_sb[0:H, off:off + cols], msk[:, :cols])
                        nc.vector.tensor_reduce(
                            out=dens[0:H, q:q + 1],
                            in_=p_sb[0:H, off:off + cols],
                            axis=mybir.AxisListType.X, op=mybir.AluOpType.add)
                    else:
                        nc.scalar.activation(
                            out=p_sb[0:H, off:off + cols], in_=sc[:, :cols],
                            func=Exp, accum_out=dens[0:H, q:q + 1])
                    # token-major p: one multi-block DVE stream transpose
                    # (16 32x32 blocks in a single instruction), then 4
                    # strided group copies scatter the 32-token blocks to
                    # their partition groups.  Cuts the serial exp->pt chain
                    # from ~3.2us to ~1.2us so the PE never starves.
                    pt_a = pta_pool.tile([32, QCH, 4, 32], bf16, tag="pta")
                    nc.vector.transpose(
                        out=pt_a.rearrange("p c g h -> p (c g h)")[:, 0:cols],
                        in_=p_sb[0:32, off:off + cols])
                    for g in range(4):
                        nc.vector.tensor_copy(
                            out=pt_t[32 * g:32 * g + 32,
                                     q * QCH:q * QCH + qc, 0:H],
                            in_=pt_a[:, 0:qc, g, 0:H])
                    return (seg, q, qc)

                def emit_num(job):
                    (seg, q, qc) = job
                    (xt_q, xn_q) = qtiles.pop((seg, q))
                    (p_sb, pt_t, dens) = segst[seg]
                    if q == 0:
                        pnums[seg] = ps_n_pool.tile(
                            [H, D], f32, tag="pnum", name="pnum")
                    pnum = pnums[seg]
                    for c in range(q * QCH, q * QCH + qc):
                        cq = c - q * QCH
                        for n0, n1 in ((0, 512), (512, 1024), (1024, D)):
                            nc.tensor.matmul(
                                pnum[:, n0:n1],
                                lhsT=pt_t[:, c, 0:H],
                                rhs=xn_q[:, cq, n0:n1],
                                start=(c == 0), stop=(c == K - 1),
                                skip_group_check=True)
                    if q == NQ - 1:
                        # drain numerators PSUM->SBUF on the mostly-idle ACT
                        # engine (keeps DVE free for the p-transpose chain)
                        onum_sb = small_pool.tile([H, D], f32, tag="onum")
                        nc.scalar.activation(
                            out=onum_sb, in_=pnum,
                            func=mybir.ActivationFunctionType.Copy)
                        nc.gpsimd.dma_start(
                            out=onum[seg * H:(seg + 1) * H, :], in_=onum_sb)
                        nc.gpsimd.dma_start(
                            out=oden[seg * H:(seg + 1) * H, :],
                            in_=dens[0:H, :])
                        del pnums[seg]
                        del segst[seg]

                jobs = [(seg, q) for seg in range(S) for q in range(NQ)]
                pending = None
                nload = min(LOOK, len(jobs))
                emit_warm_loads(jobs, nload)
                for qi, (seg, q) in enumerate(jobs):
                    while nload < len(jobs) and nload <= qi + LOOK:
                        emit_quarter_loads(*jobs[nload])
                        nload += 1
                    job = emit_scores(seg, q)
                    if pending is not None:
                        emit_num(pending)
                    pending = job
                emit_num(pending)
    nc.finalize()
    return nc


def _plan(cu_lens):
    """Host-side sharding plan. assignments[core] = [(slot, seg, start, end)]."""
    cu = [int(v) for v in cu_lens]
    n = len(cu) - 1
    lens = [cu[i + 1] - cu[i] for i in range(n)]
    S = _ceil_div(n, NCORES)
    max_len = max(lens) if lens else 1
    K = max(1, _ceil_div(max_len, P))
    use_mask = (n != S * NCORES) or any(l != K * P for l in lens)
    # fp8e3m4 values are safe when every segment pools >= ~1536 tokens
    xn_lowp = bool(lens) and min(lens) >= 1536
    assignments = []
    for i in range(NCORES):
        rows = []
        for s in range(S):
            seg = i * S + s
            if seg < n:
                rows.append((s, seg, cu[seg], cu[seg + 1]))
        assignments.append(rows)
    return S, K, assignments, use_mask, xn_lowp


def _tile_host(block_t, block_n, NQ):
    """Pre-tile one slot into the per-quarter device SBUF layouts.

    block_t: [D, L] (d-major), block_n: [L, D] (token-major), L = NQ*QP.
    Returns xt_slot [NQ, P, DC*QP], xn_slot [NQ, P, QCH*D].
    """
    xt_s = block_t.reshape(DC, P, NQ, QP).transpose(2, 1, 0, 3)
    xn_s = block_n.reshape(NQ, QCH, P, D).transpose(0, 2, 1, 3)
    return (xt_s.reshape(NQ, P, DC * QP), xn_s.reshape(NQ, P, QCH * D))


def prepare(cls, embed, cu_lens, W_k):
    """Host-side: fold wq, build both embed layouts per core, build program."""
    import ml_dtypes
    bf16 = ml_dtypes.bfloat16
    f83 = ml_dtypes.float8_e3m4

    cls = np.asarray(cls, dtype=np.float64).reshape(D)
    embed = np.asarray(embed, dtype=np.float32)
    W_k = np.asarray(W_k, dtype=np.float64)
    cu = np.asarray(cu_lens).astype(np.int64)
    n = cu.shape[0] - 1

    S, K, assignments, use_mask, xn_lowp = _plan(cu)
    NQ = _ceil_div(K, QCH)
    L = NQ * QP
    nc = _build_program(S, K, use_mask, xn_lowp)
    xn_np_dt = f83 if xn_lowp else bf16

    # wq[i, h] = scale * sum_{j in head h} cls[j] W_k[j, i]
    scale = 1.0 / math.sqrt(DH)
    wq = np.einsum("hj,hji->ih", cls.reshape(H, DH),
                   W_k.reshape(H, DH, D)) * scale
    # device layout [P, DC*H]: partition p, chunk dc holds wq[dc*P + p, :]
    wq_bf = (wq.astype(np.float32).astype(bf16)
             .reshape(DC, P, H).transpose(1, 0, 2).reshape(P, DC * H))

    emb_bf = embed.astype(bf16)
    emb_lp = embed.astype(xn_np_dt)

    in_maps = []
    for i in range(NCORES):
        rows = assignments[i]
        xt_np = np.zeros((S, NQ, P, DC * QP), dtype=bf16)
        xn_np = np.zeros((S, NQ, P, QCH * D), dtype=xn_np_dt)
        mask = np.zeros((S * L,), dtype=np.float32) if use_mask else None
        for (s, _seg, start, end) in rows:
            ln = end - start
            bt = np.zeros((D, L), dtype=bf16)
            bn = np.zeros((L, D), dtype=xn_np_dt)
            bt[:, :ln] = emb_bf[start:end].T
            bn[:ln] = emb_lp[start:end]
            xt_np[s], xn_np[s] = _tile_host(bt, bn, NQ)
            if use_mask:
                mask[s * L:s * L + ln] = 1.0
        m = {"xt": xt_np, "xn": xn_np, "wqd": wq_bf}
        if use_mask:
            m["maskin"] = mask
        in_maps.append(m)
    return nc, in_maps, assignments, n


def gather(results, assignments, n):
    head = np.arange(D) // DH
    full = np.zeros((n, 1, D), dtype=np.float32)
    for i in range(NCORES):
        onum = np.asarray(results[i]["onum"])      # (S*H, D)
        oden = np.asarray(results[i]["oden"])      # (S*H, NQ)
        for (s, seg, _start, _end) in assignments[i]:
            num = onum[s * H:(s + 1) * H, :]
            den = oden[s * H:(s + 1) * H, :].sum(axis=1)
            full[seg, 0, :] = num[head, np.arange(D)] / den[head]
    return full


def kernel(cls, embed, cu_lens, max_len, W_k, b_k):
    from concourse.bass_utils import run_bass_kernel_spmd

    nc, in_maps, assignments, n = prepare(cls, embed, cu_lens, W_k)
    res = run_bass_kernel_spmd(nc, in_maps, core_ids=list(range(NCORES)))
    return gather(res.results, assignments, n)

